# revision 103
# baseline (speedup 1.0000x reference)
"""Multi-head causal attention (b=2, t=2048, d=1024, H=16, hd=64) on 8 TRN2 cores.

Sharding: tensor-parallel over heads - 2 heads per core. Each core:
  * projects x against its Wq/Wk/Wv column slice (128 dims = 2 heads)
  * runs causal attention for its 2 heads (scores kept transposed
    [k, q] so the ctx matmul contracts k on partitions; softmax row-sums
    come for free from a ones-column appended to V)
  * multiplies ctx by its Wo row slice -> a partial [4096, 1024] output
Host sums the 8 partials and adds the bias.

This revision (vs the f32r baseline, 201.7us -> 147.2us TimelineSim):
  * all on-chip activations/weights/partials in bf16 (x/W converted
    host-side), halving DMA traffic and SBUF footprint; matmuls run bf16
    at full rate with f32 PSUM accumulation
  * V is projected directly in [token, vdim] layout (lhsT = x chunk) so
    no PE transposes / staging copies are needed
  * exact causal column offsets (bf16 has no N>=256 rate cliff)
  * single software-pipelined pass: each attention stripe carries one
    projection stripe + deferred out-projection tiles as PE filler units,
    spread evenly between block-pairs, so the PE rarely starves (p-state
    stays near 2.4 GHz); PSUM uses all 8 banks (ctx0 + ctx1 + 2x spair +
    pjA + pjB rotating proj/outproj/broadcast slots; same-PSUM-tile
    accesses serialize cross-engine, hence the pjA/pjB split)
  * PSUM evictions split across ACT (q, rb) and DVE (k, v, ctx, out);
    causal masking via affine_select on the otherwise idle GpSimd
"""

import sys
from collections import deque

for _p in ("/opt/trn_rl_repo",):
    if _p not in sys.path:
        sys.path.insert(0, _p)

import numpy as np

import concourse.bass as bass
import concourse.tile as tile
from concourse import mybir
from concourse import bass_utils

F32 = mybir.dt.float32
F32R = mybir.dt.float32r
BF16 = mybir.dt.bfloat16

P = 128          # partitions
B = 2            # batch
T = 2048         # seq len
NT = B * T       # 4096 tokens
DIN = 1024       # model dim
HD = 64          # head dim
NDC = DIN // P   # 8 d_in chunks
NSPB = T // 512  # 4 token stripes per batch
NS = B * NSPB    # 8 stripes total
VN_W = 130       # vn cols per tok-tile: [h0 dims+ones, h1 dims+ones]
NTT = NT // P    # 32 token tiles

N_CORES = 8


def _split_multi_waits(nc, max_waits=1):
    """walrus in this container caps sync waits per instruction; spill
    extra waits onto same-engine NoOps inserted right before."""
    uid = 0
    for fn in nc.m.functions:
        for blk in fn.blocks:
            insts = blk.instructions
            new_list = []
            changed = False
            for inst in insts:
                si = inst.sync_info
                ow = list(si.on_wait) if si is not None and si.on_wait else []
                if len(ow) > max_waits:
                    spill, keep = ow[:-max_waits], ow[-max_waits:]
                    for w in spill:
                        nop = mybir.InstNoOp(name=f"I-wsplit-{blk.name}-{uid}", ins=[], outs=[])
                        uid += 1
                        nop.engine = inst.engine
                        nop.sync_info = mybir.SyncInfo(on_wait=[w], on_update=[])
                        new_list.append(nop)
                    inst.sync_info = mybir.SyncInfo(
                        on_wait=keep,
                        on_update=list(si.on_update) if si.on_update else [],
                    )
                    changed = True
                new_list.append(inst)
            if changed:
                insts[:] = new_list


def build():
    nc = bass.Bass("TRN2", target_bir_lowering=False, debug=False, num_devices=N_CORES)
    # host-packed layouts (see make_in_maps):
    #   xt  [128, s, c, t]: partition = d_in within chunk c, stripe s, token t
    #   wqkv[128, c, 3, m]: partition = d_in within chunk c, m = local out-dim
    xt_d = nc.dram_tensor("xt", [P, NS * NDC * 512], BF16, kind="ExternalInput").ap()
    w_d = nc.dram_tensor("wqkv", [P, NDC * 3 * P], BF16, kind="ExternalInput").ap()
    wo_d = nc.dram_tensor("wo", [P, DIN], BF16, kind="ExternalInput").ap()
    out = nc.dram_tensor("out", [NT, DIN], BF16, kind="ExternalOutput").ap()

    EXP = mybir.ActivationFunctionType.Exp

    with tile.TileContext(nc) as tc:
        with (
            tc.tile_pool(name="const", bufs=1) as const,
            tc.tile_pool(name="persist", bufs=1) as persist,
            tc.tile_pool(name="e_p", bufs=8) as e_p,
            tc.tile_pool(name="r_p", bufs=2) as r_p,
            tc.tile_pool(name="osb_p", bufs=6) as osb_p,
            tc.tile_pool(name="ps", bufs=1, space="PSUM") as ps,
        ):
            w_sb = const.tile([P, NDC * 3 * P], BF16)
            wo_sb = const.tile([P, DIN], BF16)
            xt_sb = persist.tile([P, NS * NDC * 512], BF16)
            qkT = persist.tile([P, 2 * NT], BF16)    # [:,0:4096]=qT, [:,4096:]=kT
            vn = persist.tile([P, NTT * VN_W], BF16)
            ctxT = persist.tile([P, NT], BF16)
            ones1f = const.tile([1, P], F32)
            ones1 = const.tile([1, P], F32R)

            # staged DMAs: the first qk matmul needs only w chunk 0 + x
            # stripe-0 chunk 0; stage uploads so compute starts ~2.5us in.
            def dma(dst, src, lo, hi):
                nc.sync.dma_start(dst[:, lo:hi], src[:, lo:hi])

            SW = NDC * 512  # per-stripe xt cols
            dma(w_sb, w_d, 0, 384)
            dma(xt_sb, xt_d, 0, 512)
            dma(w_sb, w_d, 384, 4 * 384)
            dma(xt_sb, xt_d, 512, 4 * 512)
            dma(w_sb, w_d, 4 * 384, 8 * 384)
            dma(xt_sb, xt_d, 4 * 512, SW)
            dma(xt_sb, xt_d, SW, SW + SW // 2)
            dma(xt_sb, xt_d, SW + SW // 2, 2 * SW)
            dma(xt_sb, xt_d, 2 * SW, 2 * SW + SW // 2)
            dma(xt_sb, xt_d, 2 * SW + SW // 2, 3 * SW)
            nc.sync.dma_start(wo_sb[:], wo_d)
            for s in range(3, NS):
                dma(xt_sb, xt_d, s * SW, s * SW + SW // 2)
                dma(xt_sb, xt_d, s * SW + SW // 2, (s + 1) * SW)

            nc.gpsimd.memset(ones1f[:], 1.0)
            nc.vector.tensor_copy(ones1[:], ones1f[:])
            # ones columns of vn (local col 64 of each 65-wide half-slot)
            ones_view = vn[:].rearrange("p (x e) -> p x e", e=65)[:, :, 64:65]
            nc.gpsimd.memset(ones_view, 1.0)


            # ---------------- emission helpers ----------------
            region = {"late": False}

            def qk_units(s):
                """9 filler units: 8 per-chunk qk matmul pairs + evict.
                q and k land in separate 1-bank PSUM tiles so their
                evictions (ACT / DVE) run in parallel."""
                st = {}

                def qk_chunk(c):
                    def f():
                        if c == 0:
                            st["q"] = ps.tile([P, 512], F32, name=f"q{s}", tag="pjA")
                            st["k"] = ps.tile([P, 512], F32, name=f"k{s}", tag="pjB")
                        rhs = xt_sb[:, (s * NDC + c) * 512:(s * NDC + c + 1) * 512]
                        fl = dict(start=(c == 0), stop=(c == NDC - 1))
                        nc.tensor.matmul(
                            st["q"][:], w_sb[:, (c * 3) * P:(c * 3 + 1) * P], rhs, **fl)
                        nc.tensor.matmul(
                            st["k"][:], w_sb[:, (c * 3 + 1) * P:(c * 3 + 2) * P], rhs, **fl)
                    return f

                def evict():
                    nc.scalar.copy(qkT[:, s * 512:(s + 1) * 512], st["q"][:])
                    nc.vector.tensor_copy(
                        qkT[:, NT + s * 512:NT + (s + 1) * 512], st["k"][:])

                return [qk_chunk(c) for c in range(NDC)] + [evict]

            def vt_units(s):
                """4 filler units: one vT token-tile each (DVE evict on last)."""
                st = {}

                def vt_tile(tt):
                    def f():
                        if tt == 0:
                            st["pjv"] = ps.tile([P, 512], F32, name=f"vt{s}", tag="pjA")
                        pj = st["pjv"]
                        for c in range(NDC):
                            base = (s * NDC + c) * 512
                            nc.tensor.matmul(
                                pj[:, tt * P:(tt + 1) * P],
                                xt_sb[:, base + tt * P:base + (tt + 1) * P],
                                w_sb[:, (c * 3 + 2) * P:(c * 3 + 3) * P],
                                start=(c == 0), stop=(c == NDC - 1),
                            )
                        if tt == 3:
                            src = pj[:].rearrange("p (t h d) -> p t h d", t=4, h=2)
                            dstv = (
                                vn[:, s * 4 * VN_W:(s * 4 + 4) * VN_W]
                                .rearrange("p (t w) -> p t w", w=VN_W)
                                .rearrange("p t (h e) -> p t h e", h=2)[:, :, :, 0:64]
                            )
                            nc.vector.tensor_copy(dstv, src)
                    return f

                return [vt_tile(t) for t in range(4)]

            def proj_units(s):
                return qk_units(s) + vt_units(s)

            def outproj_units(b, qs):
                """4 filler units: one out-projection token-tile each;
                PSUM eviction split in halves across DVE + Pool so the pj
                slot frees in ~0.9us instead of 1.5."""
                def op_tile(gt):
                    def f():
                        pa = ps.tile([P, 512], F32, name=f"opa{gt}", tag="pjA")
                        pb = ps.tile([P, 512], F32, name=f"opb{gt}", tag="pjB")
                        lhsT = ctxT[:, gt * P:(gt + 1) * P]
                        nc.tensor.matmul(pa[:], lhsT, wo_sb[:, 0:512],
                                         start=True, stop=True)
                        nc.tensor.matmul(pb[:], lhsT, wo_sb[:, 512:1024],
                                         start=True, stop=True)
                        osb = osb_p.tile([P, DIN], BF16, name="osb")
                        nc.vector.tensor_copy(osb[:, 0:512], pa[:])
                        if region.get("tail"):
                            nc.scalar.copy(osb[:, 512:1024], pb[:])
                        else:
                            nc.vector.tensor_copy(osb[:, 512:1024], pb[:])
                        nc.sync.dma_start(out[gt * P:(gt + 1) * P, :], osb[:])
                    return f
                g0 = (b * NSPB + qs) * 4
                return [op_tile(g0 + i) for i in range(4)]

            def finalize_unit(b, q_lo, q_hi, ctx0, ctx1):
                """softmax denominators + normalization into ctxT (bf16).
                rinv is broadcast via a PE ones-matmul then staged to SBUF
                (the DVE mul may read at most one PSUM operand)."""
                qoff = b * T + q_lo
                QW = q_hi - q_lo

                def f():
                    rinv = r_p.tile([1, 1024], F32R, name="rinv")
                    with nc.allow_low_precision(reason="f32r softmax denominators"):
                        nc.vector.reciprocal(rinv[0:1, 0:QW], ctx0[64:65, 0:QW])
                        nc.vector.reciprocal(rinv[0:1, 512:512 + QW], ctx1[64:65, 0:QW])
                    rba = ps.tile([P, 512], F32, name="rba", tag="pjA")
                    rbb = ps.tile([P, 512], F32, name="rbb", tag="pjB")
                    nc.tensor.matmul(rba[0:64, 0:QW], ones1[0:1, 0:64],
                                     rinv[0:1, 0:QW], start=True, stop=True)
                    nc.tensor.matmul(rbb[0:64, 0:QW], ones1[0:1, 0:64],
                                     rinv[0:1, 512:512 + QW], start=True, stop=True)
                    rb = r_p.tile([64, 1024], F32R, name="rb")
                    nc.scalar.copy(rb[:, 0:QW], rba[0:64, 0:QW])
                    nc.vector.tensor_copy(rb[:, 512:512 + QW], rbb[0:64, 0:QW])
                    nc.vector.tensor_mul(
                        ctxT[0:64, qoff:qoff + QW], ctx0[0:64, 0:QW], rb[0:64, 0:QW])
                    nc.vector.tensor_mul(
                        ctxT[64:128, qoff:qoff + QW], ctx1[0:64, 0:QW],
                        rb[0:64, 512:512 + QW])
                return f

            def emit_attn_range(b, q_lo, q_hi, fillers, tag_sfx=""):
                """attention for queries [q_lo, q_hi) of batch b: blocks of
                128 keys, scores->exp->mask->ctx, 1-block software pipeline,
                fillers popped between ctx emissions."""
                qoff = b * T + q_lo
                QW = q_hi - q_lo
                nkb = q_hi // 128
                ctx0 = ps.tile([65, 512], F32, name=f"c0_{b}{q_lo}", tag="ctx0")
                ctx1 = ps.tile([65, 512], F32, name=f"c1_{b}{q_lo}", tag="ctx1")
                fillers = deque(fillers)
                nslots = max(q_hi // 256, 1)
                state = {"slot": 0, "popped": 0, "total": len(fillers)}
                pend = None

                def emit_ctx(item):
                    ktt, c0, N, e, kb = item
                    for h, ctx in ((0, ctx0), (1, ctx1)):
                        nc.tensor.matmul(
                            ctx[:, c0:QW],
                            vn[:, ktt * VN_W + 65 * h:ktt * VN_W + 65 * h + 65],
                            e[:, h * N:(h + 1) * N],
                            start=(kb == 0), stop=(kb == nkb - 1),
                        )

                def pop_fillers(slots_left):
                    # spread fillers evenly across the stripe's slots
                    state["slot"] += 1
                    want = (state["total"] * state["slot"] + nslots // 2) // nslots
                    while state["popped"] < want and fillers:
                        fillers.popleft()()
                        state["popped"] += 1
                    if slots_left <= 1:
                        while fillers:
                            fillers.popleft()()
                            state["popped"] += 1

                def emit_block(kb):
                    ktt = b * (T // P) + kb
                    diag = kb * 128 >= q_lo
                    c0 = max(0, kb * 128 - q_lo)
                    N = QW - c0
                    spair = ps.tile([P, 1024], F32, name="spair", tag="spair", bufs=2)
                    for h in (0, 1):
                        nc.tensor.matmul(
                            spair[:, h * 512 + c0:h * 512 + QW],
                            qkT[64 * h:64 * h + 64, NT + ktt * P:NT + (ktt + 1) * P],
                            qkT[64 * h:64 * h + 64, qoff + c0:qoff + QW],
                            start=True, stop=True,
                        )
                    e = e_p.tile([P, 1024], BF16, name="e")
                    if c0 or QW < 512:
                        nc.scalar.activation(
                            e[:, 0:2 * N].rearrange("p (h n) -> p h n", h=2),
                            spair[:].rearrange("p (h n) -> p h n", h=2)[:, :, c0:QW],
                            EXP, scale=0.125)
                    else:
                        nc.scalar.activation(e[:, 0:1024], spair[:], EXP, scale=0.125)
                    if diag:
                        # zero q < k inside the diag-crossing 128 columns
                        for h in (0, 1):
                            nc.gpsimd.affine_select(
                                out=e[:, h * N:h * N + 128],
                                in_=e[:, h * N:h * N + 128],
                                compare_op=mybir.AluOpType.is_ge,
                                fill=0.0, base=0,
                                pattern=[[1, 128]], channel_multiplier=-1,
                            )
                    return (ktt, c0, N, e, kb)

                # pairs of blocks per software-pipeline step: fewer, longer
                # PE streaks (each PE stall also costs a p-state re-ramp)
                pend = []
                for kb in range(nkb):
                    cur = emit_block(kb)
                    if kb % 2 == 1 or kb == nkb - 1:
                        for it in pend:
                            emit_ctx(it)
                        if pend:
                            pop_fillers(max((nkb - kb) // 2, 1))
                        pend = [cur] if kb % 2 == 1 else []
                        if kb == nkb - 1 and kb % 2 == 0:
                            pend = [cur]
                    else:
                        pend.append(cur)
                for it in pend:
                    emit_ctx(it)
                pop_fillers(1)
                return ctx0, ctx1

            def emit_final_stripe(b, q_lo, q_hi, fillers):
                """final stripe with progressive epilogue: as soon as the
                last block touching a 128-query column range has
                accumulated, normalize + out-project + ship that token
                tile, so only a 128-wide chain drains at the very end."""
                qoff = b * T + q_lo
                QW = q_hi - q_lo
                nkb = q_hi // 128
                ctx0 = ps.tile([65, 512], F32, name=f"f0_{b}", tag="ctx0")
                ctx1 = ps.tile([65, 512], F32, name=f"f1_{b}", tag="ctx1")
                fillers = deque(fillers)
                nslots = max(nkb // 2, 1)
                state = {"slot": 0, "popped": 0, "total": len(fillers)}

                def emit_ctx(item):
                    ktt, c0, N, e, kb = item
                    for h, ctx in ((0, ctx0), (1, ctx1)):
                        nc.tensor.matmul(
                            ctx[:, c0:QW],
                            vn[:, ktt * VN_W + 65 * h:ktt * VN_W + 65 * h + 65],
                            e[:, h * N:(h + 1) * N],
                            start=(kb == 0), stop=(kb == nkb - 1),
                        )

                def pop_fillers():
                    state["slot"] += 1
                    want = (state["total"] * state["slot"] + nslots - 1) // nslots
                    while state["popped"] < want and fillers:
                        fillers.popleft()()
                        state["popped"] += 1

                def emit_block(kb):
                    ktt = b * (T // P) + kb
                    diag = kb * 128 >= q_lo
                    c0 = max(0, kb * 128 - q_lo)
                    N = QW - c0
                    spair = ps.tile([P, 1024], F32, name="spair", tag="spair", bufs=2)
                    for h in (0, 1):
                        nc.tensor.matmul(
                            spair[:, h * 512 + c0:h * 512 + QW],
                            qkT[64 * h:64 * h + 64, NT + ktt * P:NT + (ktt + 1) * P],
                            qkT[64 * h:64 * h + 64, qoff + c0:qoff + QW],
                            start=True, stop=True,
                        )
                    e = e_p.tile([P, 1024], BF16, name="e")
                    if c0 or QW < 512:
                        nc.scalar.activation(
                            e[:, 0:2 * N].rearrange("p (h n) -> p h n", h=2),
                            spair[:].rearrange("p (h n) -> p h n", h=2)[:, :, c0:QW],
                            EXP, scale=0.125)
                    else:
                        nc.scalar.activation(e[:, 0:1024], spair[:], EXP, scale=0.125)
                    if diag:
                        for h in (0, 1):
                            nc.gpsimd.affine_select(
                                out=e[:, h * N:h * N + 128],
                                in_=e[:, h * N:h * N + 128],
                                compare_op=mybir.AluOpType.is_ge,
                                fill=0.0, base=0,
                                pattern=[[1, 128]], channel_multiplier=-1,
                            )
                    return (ktt, c0, N, e, kb)

                def fin_tile(ti, last):
                    # columns [128*ti, 128*ti+128) are final: normalize,
                    # out-project, ship.
                    lo = ti * P
                    gt = (b * T + q_lo + lo) // P
                    rinv = r_p.tile([1, 1024], F32R, name="rinv")
                    with nc.allow_low_precision(reason="f32r softmax denominators"):
                        nc.vector.reciprocal(rinv[0:1, 0:P], ctx0[64:65, lo:lo + P])
                        nc.vector.reciprocal(rinv[0:1, 512:512 + P],
                                             ctx1[64:65, lo:lo + P])
                    rba = ps.tile([P, 512], F32, name=f"frba{ti}", tag="pjA")
                    rbb = ps.tile([P, 512], F32, name=f"frbb{ti}", tag="pjB")
                    nc.tensor.matmul(rba[0:64, 0:P], ones1[0:1, 0:64],
                                     rinv[0:1, 0:P], start=True, stop=True)
                    nc.tensor.matmul(rbb[0:64, 0:P], ones1[0:1, 0:64],
                                     rinv[0:1, 512:512 + P], start=True, stop=True)
                    rb = r_p.tile([64, 1024], F32R, name="rb")
                    nc.scalar.copy(rb[:, 0:P], rba[0:64, 0:P])
                    nc.vector.tensor_copy(rb[:, 512:512 + P], rbb[0:64, 0:P])
                    nc.vector.tensor_mul(
                        ctxT[0:64, qoff + lo:qoff + lo + P],
                        ctx0[0:64, lo:lo + P], rb[0:64, 0:P])
                    nc.vector.tensor_mul(
                        ctxT[64:128, qoff + lo:qoff + lo + P],
                        ctx1[0:64, lo:lo + P], rb[0:64, 512:512 + P])
                    pa = ps.tile([P, 512], F32, name=f"fopa{ti}", tag="pjA")
                    pb = ps.tile([P, 512], F32, name=f"fopb{ti}", tag="pjB")
                    lhsT = ctxT[:, gt * P:(gt + 1) * P]
                    nc.tensor.matmul(pa[:], lhsT, wo_sb[:, 0:512],
                                     start=True, stop=True)
                    nc.tensor.matmul(pb[:], lhsT, wo_sb[:, 512:1024],
                                     start=True, stop=True)
                    osb = osb_p.tile([P, DIN], BF16, name="osb")
                    nc.vector.tensor_copy(osb[:, 0:512], pa[:])
                    nc.scalar.copy(osb[:, 512:1024], pb[:])
                    if not last:
                        nc.sync.dma_start(out[gt * P:(gt + 1) * P, :], osb[:])
                    else:
                        nc.sync.dma_start(out[gt * P:(gt + 1) * P, 0:512],
                                          osb[:, 0:512])
                        nc.sync.dma_start(out[gt * P:(gt + 1) * P, 512:1024],
                                          osb[:, 512:1024])

                ntile = QW // P
                # paired pipeline until the diag region, then single-block
                # with progressive per-tile epilogue
                sw = nkb - ntile   # first block whose completion finalizes a tile
                pend = []
                for kb in range(nkb):
                    cur = emit_block(kb)
                    if kb < sw:
                        if kb % 2 == 1:
                            for it in pend:
                                emit_ctx(it)
                            if pend:
                                pop_fillers()
                            pend = [cur]
                        else:
                            pend.append(cur)
                    else:
                        for it in pend:
                            emit_ctx(it)
                            if it[4] >= sw:
                                fin_tile(it[4] - sw, last=False)
                        if pend:
                            pop_fillers()
                        pend = [cur]
                for it in pend:
                    emit_ctx(it)
                    if it[4] >= sw:
                        fin_tile(it[4] - sw, last=(it[4] == nkb - 1))
                while fillers:
                    fillers.popleft()()

            def tail_range(b, q_lo, q_hi, ctx0, ctx1, reserved=()):
                """final query range: normalization + out-projection;
                reserved op units from earlier stripes fill the PE while the
                normalize chain drains on DVE."""
                qoff = b * T + q_lo
                QW = q_hi - q_lo
                ntile = QW // P
                rinv = r_p.tile([1, 1024], F32R, name="rinv")
                with nc.allow_low_precision(reason="f32r softmax denominators"):
                    nc.vector.reciprocal(rinv[0:1, 0:QW], ctx0[64:65, 0:QW])
                    nc.vector.reciprocal(rinv[0:1, 512:512 + QW], ctx1[64:65, 0:QW])
                region["tail"] = True
                for u in reserved:
                    u()
                rba = ps.tile([P, 512], F32, name="rba", tag="pjA")
                rbb = ps.tile([P, 512], F32, name="rbb", tag="pjB")
                nc.tensor.matmul(rba[0:64, 0:QW], ones1[0:1, 0:64],
                                 rinv[0:1, 0:QW], start=True, stop=True)
                nc.tensor.matmul(rbb[0:64, 0:QW], ones1[0:1, 0:64],
                                 rinv[0:1, 512:512 + QW], start=True, stop=True)
                rb = r_p.tile([64, 1024], F32R, name="rb")
                nc.scalar.copy(rb[:, 0:QW], rba[0:64, 0:QW])
                nc.vector.tensor_copy(rb[:, 512:512 + QW], rbb[0:64, 0:QW])
                g0 = (b * T + q_lo) // P
                for ti in range(ntile):
                    lo = ti * P
                    nc.vector.tensor_mul(
                        ctxT[0:64, qoff + lo:qoff + lo + P],
                        ctx0[0:64, lo:lo + P], rb[0:64, lo:lo + P])
                    nc.vector.tensor_mul(
                        ctxT[64:128, qoff + lo:qoff + lo + P],
                        ctx1[0:64, lo:lo + P], rb[0:64, 512 + lo:512 + lo + P])
                    gt = g0 + ti
                    lhsT = ctxT[:, gt * P:(gt + 1) * P]
                    osb = osb_p.tile([P, DIN], BF16, name="osb")
                    if ti % 2 == 0 or ti == ntile - 1:
                        # even + last tiles: pjA/pjB, parallel half evictions
                        pa = ps.tile([P, 512], F32, name=f"opta{ti}", tag="pjA")
                        pb = ps.tile([P, 512], F32, name=f"optb{ti}", tag="pjB")
                        nc.tensor.matmul(pa[:], lhsT, wo_sb[:, 0:512],
                                         start=True, stop=True)
                        nc.tensor.matmul(pb[:], lhsT, wo_sb[:, 512:1024],
                                         start=True, stop=True)
                        nc.vector.tensor_copy(osb[:, 0:512], pa[:])
                        nc.scalar.copy(osb[:, 512:1024], pb[:])
                    else:
                        # middle tiles borrow the (now idle) spair banks so
                        # the pjA/pjB rotation is not the serializer
                        sp = ps.tile([P, 1024], F32, name=f"opt{ti}",
                                     tag="spair", bufs=2)
                        nc.tensor.matmul(sp[:, 0:512], lhsT, wo_sb[:, 0:512],
                                         start=True, stop=True)
                        nc.tensor.matmul(sp[:, 512:1024], lhsT, wo_sb[:, 512:1024],
                                         start=True, stop=True)
                        eng = nc.scalar.copy if ti % 4 == 1 else nc.vector.tensor_copy
                        eng(osb[:], sp[:].rearrange("p (i n) -> p i n", i=2))
                    if ti < ntile - 1:
                        nc.sync.dma_start(out[gt * P:(gt + 1) * P, :], osb[:])
                    else:
                        nc.sync.dma_start(out[gt * P:(gt + 1) * P, 0:512],
                                          osb[:, 0:512])
                        nc.sync.dma_start(out[gt * P:(gt + 1) * P, 512:1024],
                                          osb[:, 512:1024])

            # ---------------- main schedule ----------------
            # Natural batch order: every attention stripe carries exactly one
            # projection-stripe feed (A(1,qs) only needs b=1 stripes <= qs),
            # so PE filler work is spread uniformly across the whole run.
            attn_list = [(0, 1), (0, 2), (0, 3), (1, 0), (1, 1), (1, 2)]
            feeds = {(0, 1): [2], (0, 2): [3], (0, 3): [4, 5],
                     (1, 0): [6], (1, 1): [7]}
            op_pops = {(0, 1): 2, (0, 2): 2, (0, 3): 4,
                       (1, 0): 0, (1, 1): 4, (1, 2): 8}

            for u in proj_units(0):
                u()
            c0_, c1_ = emit_attn_range(0, 0, 512, proj_units(1))
            fin_carry = finalize_unit(0, 0, 512, c0_, c1_)
            op_queue = deque(outproj_units(0, 0))

            for (b, qs) in attn_list:
                fillers = []
                if fin_carry is not None:
                    fillers.append(fin_carry)
                    fin_carry = None
                for _ in range(min(op_pops[(b, qs)], len(op_queue))):
                    fillers.append(op_queue.popleft())
                for s in feeds.get((b, qs), []):
                    fillers += proj_units(s)
                q_lo, q_hi = qs * 512, qs * 512 + 512
                ctx0, ctx1 = emit_attn_range(b, q_lo, q_hi, fillers)
                fin_carry = finalize_unit(b, q_lo, q_hi, ctx0, ctx1)
                op_queue.extend(outproj_units(b, qs))

            # final stripe (1, 3)
            fillers = [fin_carry]
            for _ in range(min(6, len(op_queue))):
                fillers.append(op_queue.popleft())
            ca0, ca1 = emit_attn_range(1, 1536, 2048, fillers)
            tail_range(1, 1536, 2048, ca0, ca1, reserved=list(op_queue))

    _split_multi_waits(nc)
    return nc


_NC_CACHE = None


def _get_nc():
    global _NC_CACHE
    if _NC_CACHE is None:
        _NC_CACHE = build()
    return _NC_CACHE


def make_in_maps(x, Wq, Wk, Wv, Wo):
    import ml_dtypes
    bf16 = ml_dtypes.bfloat16

    xt = np.asarray(x, np.float32).reshape(NT, DIN).T          # [1024, 4096]
    xt_packed = np.ascontiguousarray(
        xt.reshape(NDC, P, NS, 512).transpose(1, 2, 0, 3).reshape(P, NS * NDC * 512)
    ).astype(bf16)

    in_maps = []
    for core in range(N_CORES):
        sl = slice(core * P, (core + 1) * P)
        wq = np.asarray(Wq, np.float32)[:, sl].reshape(NDC, P, P)
        wk = np.asarray(Wk, np.float32)[:, sl].reshape(NDC, P, P)
        wv = np.asarray(Wv, np.float32)[:, sl].reshape(NDC, P, P)
        wqkv = np.ascontiguousarray(
            np.stack([wq, wk, wv], axis=1).transpose(2, 0, 1, 3).reshape(P, NDC * 3 * P)
        ).astype(bf16)
        wo = np.ascontiguousarray(np.asarray(Wo, np.float32)[sl, :]).astype(bf16)
        in_maps.append({"xt": xt_packed, "wqkv": wqkv, "wo": wo})
    return in_maps


def kernel(x, Wq, Wk, Wv, Wo, bo):
    nc = _get_nc()
    in_maps = make_in_maps(x, Wq, Wk, Wv, Wo)
    res = bass_utils.run_bass_kernel_spmd(
        nc, in_maps, core_ids=list(range(N_CORES)), trace=False
    )
    acc = np.zeros((NT, DIN), dtype=np.float64)
    for r in res.results:
        acc += r["out"].astype(np.float64)
    acc += np.asarray(bo, dtype=np.float64)[None, :]
    return acc.astype(np.float32).reshape(B, T, DIN)


if __name__ == "__main__":
    rng = np.random.default_rng(0)
    x = rng.standard_normal((B, T, DIN)).astype(np.float32)
    Wq = rng.standard_normal((DIN, DIN)).astype(np.float32) * 0.02
    Wk = rng.standard_normal((DIN, DIN)).astype(np.float32) * 0.02
    Wv = rng.standard_normal((DIN, DIN)).astype(np.float32) * 0.02
    Wo = rng.standard_normal((DIN, DIN)).astype(np.float32) * 0.02
    bo = np.zeros(DIN, dtype=np.float32)
    out = kernel(x=x, Wq=Wq, Wk=Wk, Wv=Wv, Wo=Wo, bo=bo)
    print("out", out.shape, out.dtype, float(np.abs(out).max()))
